# revision 8
# baseline (speedup 1.0000x reference)
"""Trainium2 Bass kernel for nn_EpochMixer: 2-layer post-norm transformer over
B*T independent 4-token epochs (CLS + 3 modalities), d_model=128, 8 heads,
ffn=512, run data-parallel over 8 NeuronCores (4096 epochs / 16384 tokens per
core).

v2 design notes (per core):
  - Residual stream x feature-major [128 d, 16384 tokens] in BF16; all PE
    matmuls bf16 (FWL fast weight loads), psum accumulation f32 for the
    residual adds (Wo/W2 + identity-matmul residual), single-shot matmuls
    (QKV, W1) write bf16 psum directly so evacuations run at DVE 2x rate.
  - Attention token-major: QKV with x-subchunk stationary; k/v partition-
    rotated within 4-token epochs by SBUF->SBUF DMAs; scores = bf16 multiply
    + log-tree adds (avoids 1x tensor_reduce); softmax normalization deferred:
    AV accumulates unnormalized exp-scores, one final multiply by 1/den.
  - ao token-major -> feature-major via DMA-engine XBAR transposes (off PE).
  - LayerNorm: per-token (sum, sumsq) via tiny stats matmuls (bf16 data
    stationary, ones column moving), math in [token, subchunk] layout,
    rstd/-mu*rstd rows bounced through DRAM into [1, NTOK] row tiles,
    partition-broadcast via K=1 matmuls (ones-row stationary) into bf16 psum,
    apply = STT mult + TT add on DVE.
  - Squares for LN stats on GpSimd; residual-sum evacuations on ACT.
"""

import numpy as np
import ml_dtypes

B, T, M, D, H, L, F = 16, 2048, 3, 128, 8, 2, 512
S = M + 1
DH = D // H
NCORE = 8
EPC = B * T // NCORE          # 4096 epochs per core
NTOK = EPC * S                # 16384 tokens per core
CH = 512
NCH = NTOK // CH              # 32
NSUB = CH // 128              # 4
NS = NCH * NSUB               # 128 subchunks
GRP = 4                       # chunks per attention group
NGRP = NCH // GRP
EPS = 1e-5

_BUILD_CACHE = {}


def _split_ctrl_waits(nc, mybir):
    """walrus here rejects >1 SyncWait per instruction: hoist extras onto
    single-wait NOPs inserted in front."""
    ctr = 0
    fn = nc.m.functions[0]
    for bb in fn.blocks:
        new_insts = []
        for ins in bb.instructions:
            si = getattr(ins, "sync_info", None)
            waits = list(si.on_wait) if si is not None and si.on_wait else []
            if len(waits) > 1:
                for w in waits[1:]:
                    ctr += 1
                    nop = mybir.InstNoOp(name=f"WSPLIT-{ctr}", ins=[], outs=[])
                    nop.engine = ins.engine
                    nop.sync_info = mybir.SyncInfo(on_wait=[w], on_update=[])
                    new_insts.append(nop)
                ins.sync_info = mybir.SyncInfo(
                    on_wait=waits[:1], on_update=list(si.on_update or []))
            new_insts.append(ins)
        bb.instructions = new_insts


def _build_program():
    import contextlib
    import concourse.bass as bass
    import concourse.tile as tile
    from concourse import mybir

    f32 = mybir.dt.float32
    bf16 = mybir.dt.bfloat16
    AF = mybir.ActivationFunctionType
    ALU = mybir.AluOpType
    SCALE = 1.0 / np.sqrt(DH)

    nc = bass.Bass()

    x0_d = nc.declare_dram_parameter("x0", [128, NTOK], bf16, isOutput=False)
    wq_d = [nc.declare_dram_parameter(f"wqkv{l}", [128, 3 * D], bf16,
                                      isOutput=False) for l in range(L)]
    wo_d = [nc.declare_dram_parameter(f"wo{l}", [128, D], bf16, isOutput=False)
            for l in range(L)]
    w1_d = [nc.declare_dram_parameter(f"w1{l}", [128, F], bf16, isOutput=False)
            for l in range(L)]
    w2_d = [nc.declare_dram_parameter(f"w2{l}", [128, F], bf16, isOutput=False)
            for l in range(L)]
    idb_d = nc.declare_dram_parameter("identb", [128, 128], bf16,
                                      isOutput=False)
    y_d = nc.declare_dram_parameter("y", [EPC, 128], f32, isOutput=True)

    # DRAM bounce for LN row reshuffle: [token-sub major] -> [1, NTOK] rows
    scr = nc.dram_tensor("scr_rows", [4 * L, NTOK], bf16)

    with tile.TileContext(nc) as tc, contextlib.ExitStack() as top:
        consts = top.enter_context(tc.tile_pool(name="consts", bufs=1))
        resid = top.enter_context(tc.tile_pool(name="resid", bufs=1))

        epst = consts.tile([128, 1], f32, tag="epst")
        nc.vector.memset(epst, EPS)
        ones1 = consts.tile([128, 1], bf16, tag="ones1")
        nc.vector.memset(ones1, 1.0)
        onesrow = consts.tile([1, 128], bf16, tag="onesrow")
        nc.vector.memset(onesrow, 1.0)
        ident_b = consts.tile([128, 128], bf16, tag="identb")
        nc.gpsimd.dma_start(out=ident_b, in_=idb_d[:, :])

        wq_b, wo_b, w1_b, w2_b = [], [], [], []
        for l in range(L):
            t = consts.tile([128, 3 * D], bf16, tag=f"wqb{l}")
            nc.gpsimd.dma_start(out=t, in_=wq_d[l][:, :])
            wq_b.append(t)
            t = consts.tile([128, D], bf16, tag=f"wob{l}")
            nc.gpsimd.dma_start(out=t, in_=wo_d[l][:, :])
            wo_b.append(t)
            t = consts.tile([128, F], bf16, tag=f"w1b{l}")
            nc.gpsimd.dma_start(out=t, in_=w1_d[l][:, :])
            w1_b.append(t)
            t = consts.tile([128, F], bf16, tag=f"w2b{l}")
            nc.gpsimd.dma_start(out=t, in_=w2_d[l][:, :])
            w2_b.append(t)

        # residual stream: direct bf16 load, chunked so compute starts early
        x = resid.tile([128, NTOK], bf16)
        for c in range(8):
            w = NTOK // 8
            nc.sync.dma_start(out=x[:, c * w:(c + 1) * w],
                              in_=x0_d[:, c * w:(c + 1) * w])

        rows_rstd = resid.tile([1, NTOK], bf16, tag="rows_rstd")
        rows_nmur = resid.tile([1, NTOK], bf16, tag="rows_nmur")

        def ln_finish(ln_idx, stats_all, scpool, pspool):
            """stats_all [128 tok-in-sub, NCH, NSUB, 2]=(sum, sumsq) ->
            rstd/-mu*rstd rows -> K=1 broadcast matmuls -> x = x*r + m."""
            sview = stats_all.rearrange("p c s t -> p (c s) t")
            s0 = sview[:, :, 0]
            s1 = sview[:, :, 1]
            t1 = scpool.tile([128, NS], f32, tag="ln_t1")
            nc.vector.tensor_tensor(out=t1, in0=s0, in1=s0, op=ALU.mult)
            v = scpool.tile([128, NS], f32, tag="ln_v")
            nc.vector.scalar_tensor_tensor(
                out=v, in0=t1, scalar=-1.0 / 128.0, in1=s1,
                op0=ALU.mult, op1=ALU.add)
            lnv = scpool.tile([128, NS], f32, tag="ln_lnv")
            nc.scalar.activation(out=lnv, in_=v, func=AF.Ln,
                                 scale=1.0 / 128.0, bias=epst)
            rstd_f = scpool.tile([128, NS], f32, tag="ln_rstdf")
            nc.scalar.activation(out=rstd_f, in_=lnv, func=AF.Exp, scale=-0.5)
            rstd_b = scpool.tile([128, NS], bf16, tag="ln_rstdb")
            nc.vector.tensor_copy(out=rstd_b, in_=rstd_f)
            nmur_b = scpool.tile([128, NS], bf16, tag="ln_nmurb")
            nc.vector.scalar_tensor_tensor(
                out=nmur_b, in0=s0, scalar=-1.0 / 128.0, in1=rstd_f,
                op0=ALU.mult, op1=ALU.mult)

            # PE-transpose to [s, t] so the DRAM bounce is contiguous
            # (token_global = s*128 + t), then read back as [1, NTOK] rows.
            for src_t, slot, rows in ((rstd_b, 2 * ln_idx, rows_rstd),
                                      (nmur_b, 2 * ln_idx + 1, rows_nmur)):
                ptl = pspool.tile([128, 128], bf16, tag="lnpt")
                nc.tensor.matmul(ptl, src_t, ident_b, is_transpose=True)
                tsb = scpool.tile([128, 128], bf16, tag="lntsb")
                nc.vector.tensor_copy(out=tsb, in_=ptl)
                nc.sync.dma_start(
                    out=scr[slot].rearrange("(s t) -> s t", t=128), in_=tsb)
                nc.sync.dma_start(
                    out=rows,
                    in_=bass.AP(tensor=scr, offset=slot * NTOK,
                                ap=[[0, 1], [1, NTOK]]))

            for c in range(NCH):
                t0 = c * CH
                rep1 = pspool.tile([128, CH], f32, tag="rep1")
                nc.tensor.matmul(rep1, onesrow,
                                 rows_rstd[0:1, t0:t0 + CH],
                                 start=True, stop=True)
                rep2 = pspool.tile([128, CH], f32, tag="rep2")
                nc.tensor.matmul(rep2, onesrow,
                                 rows_nmur[0:1, t0:t0 + CH],
                                 start=True, stop=True)
                nc.vector.scalar_tensor_tensor(
                    out=x[:, t0:t0 + CH], in0=x[:, t0:t0 + CH], scalar=0.0,
                    in1=rep1, op0=ALU.bypass, op1=ALU.mult)
                nc.vector.tensor_tensor(
                    out=x[:, t0:t0 + CH], in0=x[:, t0:t0 + CH], in1=rep2,
                    op=ALU.add)

        for l in range(L):
            # =================== attention ===================
            GSUB = GRP * NSUB  # subchunks per group
            with contextlib.ExitStack() as actx:
                up = actx.enter_context(tc.tile_pool(name=f"st{l}", bufs=1))
                stats_all = up.tile([128, NCH, NSUB, 2], f32, tag="stats_all")
                gctx = actx.enter_context(contextlib.ExitStack())
                ap_ = gctx.enter_context(
                    tc.tile_pool(name=f"qkv{l}", bufs=2))
                apbig = gctx.enter_context(
                    tc.tile_pool(name=f"qkvb{l}", bufs=1))
                sctch = gctx.enter_context(
                    tc.tile_pool(name=f"asc{l}", bufs=1))
                qkps = gctx.enter_context(
                    tc.tile_pool(name=f"qkps{l}", bufs=1, space="PSUM"))
                ps4 = gctx.enter_context(
                    tc.tile_pool(name=f"ps4{l}", bufs=1, space="PSUM"))
                ev4 = gctx.enter_context(
                    tc.tile_pool(name=f"ev4{l}", bufs=3))
                for g in range(NGRP):
                    c0 = g * GRP
                    with contextlib.ExitStack() as s1:
                        q_all = ap_.tile([128, GSUB, 128], bf16, tag="q_all")
                        k_all = ap_.tile([128, GSUB, 128], bf16, tag="k_all")
                        v_all = ap_.tile([128, GSUB, 128], bf16, tag="v_all")
                        kS = apbig.tile([128, 3, GSUB, 128], bf16, tag="kS")
                        vS = apbig.tile([128, 3, GSUB, 128], bf16, tag="vS")
                        ao_tm = ap_.tile([128, GSUB, 128], bf16, tag="ao_tm")

                        # ---- A1: token-major qkv
                        if True:
                            for ci in range(GRP):
                                c = c0 + ci
                                pq = qkps.tile([128, NSUB, 512], f32,
                                               tag="pqkv")
                                for s4 in range(NSUB):
                                    t0 = c * CH + s4 * 128
                                    nc.tensor.matmul(pq[:, s4, 0:3 * D],
                                                     x[:, t0:t0 + 128],
                                                     wq_b[l],
                                                     start=True, stop=True)
                                csl = slice(ci * NSUB, (ci + 1) * NSUB)
                                nc.vector.tensor_copy(out=q_all[:, csl, :],
                                                      in_=pq[:, :, 0:128])
                                nc.scalar.activation(out=k_all[:, csl, :],
                                                     in_=pq[:, :, 128:256],
                                                     func=AF.Copy)
                                nc.vector.tensor_copy(out=v_all[:, csl, :],
                                                      in_=pq[:, :, 256:384])

                        # ---- A2: partition rotations ----
                        for dlt in (1, 2, 3):
                            for s in range(S):
                                sp = (s + dlt) % S
                                for srct, dstt in ((k_all, kS), (v_all, vS)):
                                    s_ap = srct.rearrange(
                                        "(gg s) c f -> s gg c f", s=S)[sp]
                                    d_ap = dstt[:, dlt - 1].rearrange(
                                        "(gg s) c f -> s gg c f", s=S)[s]
                                    nc.sync.dma_start(out=d_ap, in_=s_ap)

                        # ---- A3: scores / softmax / AV ----
                        stt = sctch.tile([128, 4, GSUB, H], bf16, tag="stt")
                        for dlt in range(4):
                            kk = k_all if dlt == 0 else kS[:, dlt - 1]
                            pp = sctch.tile([128, GSUB, H, DH], bf16,
                                            tag="pp")
                            nc.vector.tensor_tensor(
                                out=pp,
                                in0=q_all.rearrange("p c (h d) -> p c h d",
                                                    h=H),
                                in1=kk.rearrange("p c (h d) -> p c h d", h=H),
                                op=ALU.mult)
                            r8 = sctch.tile([128, GSUB, H, 8], bf16, tag="r8")
                            nc.vector.tensor_tensor(
                                out=r8, in0=pp[:, :, :, 0:8],
                                in1=pp[:, :, :, 8:16], op=ALU.add)
                            r4 = sctch.tile([128, GSUB, H, 4], bf16, tag="r4")
                            nc.vector.tensor_tensor(
                                out=r4, in0=r8[:, :, :, 0:4],
                                in1=r8[:, :, :, 4:8], op=ALU.add)
                            r2 = sctch.tile([128, GSUB, H, 2], bf16, tag="r2")
                            nc.vector.tensor_tensor(
                                out=r2, in0=r4[:, :, :, 0:2],
                                in1=r4[:, :, :, 2:4], op=ALU.add)
                            nc.vector.tensor_tensor(
                                out=stt[:, dlt], in0=r2[:, :, :, 0],
                                in1=r2[:, :, :, 1], op=ALU.add)
                        ee = sctch.tile([128, 4, GSUB, H], bf16, tag="ee")
                        nc.scalar.activation(out=ee, in_=stt, func=AF.Exp,
                                             scale=SCALE)
                        d01 = sctch.tile([128, GSUB, H], bf16, tag="d01")
                        nc.vector.tensor_tensor(out=d01, in0=ee[:, 0],
                                                in1=ee[:, 1], op=ALU.add)
                        d23 = sctch.tile([128, GSUB, H], bf16, tag="d23")
                        nc.vector.tensor_tensor(out=d23, in0=ee[:, 2],
                                                in1=ee[:, 3], op=ALU.add)
                        den = sctch.tile([128, GSUB, H], bf16, tag="den")
                        nc.vector.tensor_tensor(out=den, in0=d01, in1=d23,
                                                op=ALU.add)
                        rcp = sctch.tile([128, GSUB, H], f32, tag="rcp")
                        nc.vector.reciprocal(out=rcp, in_=den)
                        rcp_b = sctch.tile([128, GSUB, H], bf16, tag="rcp_b")
                        nc.vector.tensor_copy(out=rcp_b, in_=rcp)

                        o_sl = ao_tm.rearrange("p c (h d) -> p c h d", h=H)
                        for dlt in range(4):
                            vv = v_all if dlt == 0 else vS[:, dlt - 1]
                            e_b = ee[:, dlt].unsqueeze(3).broadcast_to(
                                (128, GSUB, H, DH))
                            if dlt == 0:
                                nc.vector.tensor_tensor(
                                    out=o_sl,
                                    in0=vv.rearrange("p c (h d) -> p c h d",
                                                     h=H),
                                    in1=e_b, op=ALU.mult)
                            else:
                                tmp = sctch.tile([128, GSUB, H, DH], bf16,
                                                 tag="avtmp")
                                nc.vector.tensor_tensor(
                                    out=tmp,
                                    in0=vv.rearrange("p c (h d) -> p c h d",
                                                     h=H),
                                    in1=e_b, op=ALU.mult)
                                nc.vector.tensor_tensor(out=o_sl, in0=o_sl,
                                                        in1=tmp, op=ALU.add)
                        nc.vector.tensor_tensor(
                            out=o_sl, in0=o_sl,
                            in1=rcp_b.unsqueeze(3).broadcast_to(
                                (128, GSUB, H, DH)),
                            op=ALU.mult)

                        # ---- A4: Wo + residual + stats ----
                        with contextlib.ExitStack() as bctx:
                            for ci in range(GRP):
                                c = c0 + ci
                                t0 = c * CH
                                ao_fm = ev4.tile([128, 512], bf16,
                                                 tag="ao_fm")
                                for s4 in range(NSUB):
                                    ptr = ps4.tile([128, 128], bf16,
                                                   tag="ptr")
                                    nc.tensor.matmul(
                                        ptr, ao_tm[:, ci * NSUB + s4, :],
                                        ident_b, is_transpose=True)
                                    nc.vector.tensor_copy(
                                        out=ao_fm[:, s4 * 128:(s4 + 1) * 128],
                                        in_=ptr)
                                pu = ps4.tile([128, 512], f32, tag="pu")
                                nc.tensor.matmul(pu, wo_b[l], ao_fm,
                                                 start=True, stop=False)
                                nc.tensor.matmul(pu, ident_b,
                                                 x[:, t0:t0 + CH],
                                                 start=False, stop=True)
                                nc.scalar.activation(out=x[:, t0:t0 + CH],
                                                     in_=pu, func=AF.Copy)
                                sq = ev4.tile([128, 512], bf16, tag="sq1")
                                nc.gpsimd.tensor_tensor(
                                    out=sq, in0=x[:, t0:t0 + CH],
                                    in1=x[:, t0:t0 + CH], op=ALU.mult)
                                pst = ps4.tile([128, NSUB, 2], f32,
                                               tag="pst1")
                                for s4 in range(NSUB):
                                    tt = t0 + s4 * 128
                                    nc.tensor.matmul(pst[:, s4, 0:1],
                                                     x[:, tt:tt + 128],
                                                     ones1,
                                                     start=True, stop=True)
                                    nc.tensor.matmul(
                                        pst[:, s4, 1:2],
                                        sq[:, s4 * 128:(s4 + 1) * 128],
                                        ones1, start=True, stop=True,
                                        skip_group_check=True)
                                nc.vector.tensor_copy(out=stats_all[:, c],
                                                      in_=pst)

                gctx.close()
                with contextlib.ExitStack() as lctx:
                    lnps = lctx.enter_context(tc.tile_pool(
                        name=f"lnps_a{l}", bufs=2, space="PSUM"))
                    lnsc = lctx.enter_context(tc.tile_pool(
                        name=f"lnsc_a{l}", bufs=1))
                    ln_finish(2 * l + 0, stats_all, lnsc, lnps)

            # =================== FFN ===================
            with contextlib.ExitStack() as fctx:
                fp = fctx.enter_context(tc.tile_pool(name=f"ffn{l}", bufs=1))
                fsc = fctx.enter_context(tc.tile_pool(name=f"fsc{l}", bufs=3))
                stats2 = fp.tile([128, NCH, NSUB, 2], f32, tag="stats2")
                with contextlib.ExitStack() as floop:
                    fps = floop.enter_context(tc.tile_pool(
                        name=f"fps{l}", bufs=2, space="PSUM"))
                    fps2 = floop.enter_context(tc.tile_pool(
                        name=f"fps2{l}", bufs=2, space="PSUM"))
                    FCH = 256
                    for hcf in range(2 * NCH):
                        t0 = hcf * FCH
                        ph = fps.tile([128, 4, FCH], f32, tag="ph")
                        for j in range(4):
                            nc.tensor.matmul(
                                ph[:, j, :],
                                w1_b[l][:, j * 128:(j + 1) * 128],
                                x[:, t0:t0 + FCH], start=True, stop=True)
                        hh = fsc.tile([128, 4, FCH], bf16, tag="hh")
                        nc.scalar.activation(out=hh, in_=ph, func=AF.Gelu)
                        pf = fps2.tile([128, FCH], f32, tag="pf")
                        for j in range(4):
                            nc.tensor.matmul(
                                pf, w2_b[l][:, j * 128:(j + 1) * 128],
                                hh[:, j, :], start=(j == 0), stop=False)
                        nc.tensor.matmul(pf, ident_b, x[:, t0:t0 + FCH],
                                         start=False, stop=True)
                        nc.scalar.activation(out=x[:, t0:t0 + FCH], in_=pf,
                                             func=AF.Copy)
                        sq = fsc.tile([128, FCH], bf16, tag="sq2")
                        nc.gpsimd.tensor_tensor(
                            out=sq, in0=x[:, t0:t0 + FCH],
                            in1=x[:, t0:t0 + FCH], op=ALU.mult)
                        pst = fps2.tile([128, 2, 2], f32, tag="pst2")
                        for s2 in range(2):
                            tt = t0 + s2 * 128
                            nc.tensor.matmul(pst[:, s2, 0:1],
                                             x[:, tt:tt + 128], ones1,
                                             start=True, stop=True)
                            nc.tensor.matmul(pst[:, s2, 1:2],
                                             sq[:, s2 * 128:(s2 + 1) * 128],
                                             ones1, start=True, stop=True,
                                             skip_group_check=True)
                        nc.vector.tensor_copy(
                            out=stats2[:, hcf // 2,
                                       (hcf % 2) * 2:(hcf % 2) * 2 + 2, :],
                            in_=pst)
                with contextlib.ExitStack() as lctx:
                    lnps = lctx.enter_context(tc.tile_pool(
                        name=f"lnps_f{l}", bufs=2, space="PSUM"))
                    lnsc = lctx.enter_context(tc.tile_pool(
                        name=f"lnsc_f{l}", bufs=1))
                    ln_finish(2 * l + 1, stats2, lnsc, lnps)

        # =================== CLS extraction ===================
        with contextlib.ExitStack() as octx:
            op_ = octx.enter_context(tc.tile_pool(name="outp", bufs=1))
            oev = octx.enter_context(tc.tile_pool(name="outev", bufs=3))
            cls_fm = op_.tile([128, EPC], bf16, tag="cls_fm")
            nc.vector.tensor_copy(
                out=cls_fm, in_=x.rearrange("p (e s) -> p e s", s=S)[:, :, 0])
            ops_ = octx.enter_context(tc.tile_pool(name="outps", bufs=2,
                                                   space="PSUM"))
            for g in range(EPC // 128):
                pt = ops_.tile([128, 128], bf16, tag="clsps")
                nc.tensor.matmul(pt, cls_fm[:, g * 128:(g + 1) * 128],
                                 ident_b, is_transpose=True)
                ot = oev.tile([128, 128], f32, tag="ot")
                nc.vector.tensor_copy(out=ot, in_=pt)
                nc.sync.dma_start(out=y_d[g * 128:(g + 1) * 128, :], in_=ot)

    _split_ctrl_waits(nc, mybir)
    return nc


def _get_program():
    if "nc" not in _BUILD_CACHE:
        _BUILD_CACHE["nc"] = _build_program()
    return _BUILD_CACHE["nc"]


# ==========================================================================
def _prepare_in_maps(inputs):
    """Build the per-core input maps from the full (unsharded) input dict."""
    Wqkv = np.asarray(inputs["Wqkv"], np.float32)
    Wo = np.asarray(inputs["Wo"], np.float32)
    W1 = np.asarray(inputs["W1"], np.float32)
    W2 = np.asarray(inputs["W2"], np.float32)
    cls = np.asarray(inputs["cls_token"], np.float32).reshape(D)

    bf = ml_dtypes.bfloat16
    zs = [np.asarray(inputs[f"z{m}"], np.float32).reshape(B * T, D)
          for m in range(M)]
    base = {
        "identb": np.eye(128, dtype=bf),
    }
    for l in range(L):
        base[f"wqkv{l}"] = np.ascontiguousarray(Wqkv[l].T).astype(bf)
        base[f"wo{l}"] = np.ascontiguousarray(Wo[l].T).astype(bf)
        base[f"w1{l}"] = np.ascontiguousarray(W1[l].T).astype(bf)
        base[f"w2{l}"] = np.ascontiguousarray(
            W2[l].T.reshape(4, 128, 128).transpose(1, 0, 2).reshape(128, 512)
        ).astype(bf)

    in_maps = []
    for c in range(NCORE):
        e0, e1 = c * EPC, (c + 1) * EPC
        x0 = np.empty((128, NTOK), np.float32)
        xv = x0.reshape(128, EPC, S)
        xv[:, :, 0] = cls[:, None]
        for m in range(M):
            xv[:, :, 1 + m] = zs[m][e0:e1].T
        in_maps.append({**base, "x0": x0.astype(bf)})
    return in_maps


def kernel(z0, z1, z2, cls_token, Wqkv, bqkv, Wo, bo, W1, b1, W2, b2,
           ln1_g, ln1_b, ln2_g, ln2_b):
    import concourse.bass_utils as bass_utils

    z0 = np.asarray(z0, np.float32)
    z1 = np.asarray(z1, np.float32)
    z2 = np.asarray(z2, np.float32)
    cls = np.asarray(cls_token, np.float32).reshape(D)
    Wqkv = np.asarray(Wqkv, np.float32)
    Wo = np.asarray(Wo, np.float32)
    W1 = np.asarray(W1, np.float32)
    W2 = np.asarray(W2, np.float32)

    # fast path exploits the module's zero biases / unit gains
    for tns, want in ((bqkv, 0), (bo, 0), (b1, 0), (b2, 0),
                      (ln1_b, 0), (ln2_b, 0), (ln1_g, 1), (ln2_g, 1)):
        if not np.allclose(np.asarray(tns, np.float32), want, atol=1e-6):
            return _numpy_fallback(
                z0, z1, z2, cls, Wqkv, np.asarray(bqkv, np.float32),
                Wo, np.asarray(bo, np.float32), W1, np.asarray(b1, np.float32),
                W2, np.asarray(b2, np.float32),
                np.asarray(ln1_g, np.float32), np.asarray(ln1_b, np.float32),
                np.asarray(ln2_g, np.float32), np.asarray(ln2_b, np.float32))

    nc = _get_program()
    in_maps = _prepare_in_maps({
        "z0": z0, "z1": z1, "z2": z2, "cls_token": cls,
        "Wqkv": Wqkv, "Wo": Wo, "W1": W1, "W2": W2,
    })

    res = bass_utils.run_bass_kernel_spmd(nc, in_maps, list(range(NCORE)))
    out = np.empty((B * T, D), np.float32)
    for c in range(NCORE):
        out[c * EPC:(c + 1) * EPC] = res.results[c]["y"]
    return out.reshape(B, T, D)


def _numpy_fallback(z0, z1, z2, cls, Wqkv, bqkv, Wo, bo, W1, b1, W2, b2,
                    g1, be1, g2, be2):
    from scipy.special import erf
    N = B * T
    z = np.stack([z0.reshape(N, D), z1.reshape(N, D), z2.reshape(N, D)], 1)
    xx = np.concatenate([np.broadcast_to(cls, (N, 1, D)), z], 1)

    def ln(v, g, b):
        mu = v.mean(-1, keepdims=True)
        var = ((v - mu) ** 2).mean(-1, keepdims=True)
        return (v - mu) / np.sqrt(var + EPS) * g + b

    for l in range(L):
        qkv = xx @ Wqkv[l].T + bqkv[l]
        q, k, v = np.split(qkv, 3, -1)
        q = q.reshape(N, S, H, DH)
        k = k.reshape(N, S, H, DH)
        v = v.reshape(N, S, H, DH)
        s = np.einsum('nihd,njhd->nhij', q, k) / np.sqrt(DH)
        e = np.exp(s - s.max(-1, keepdims=True))
        a = e / e.sum(-1, keepdims=True)
        o = np.einsum('nhij,njhd->nihd', a, v).reshape(N, S, D)
        xx = ln(xx + (o @ Wo[l].T + bo[l]), g1[l], be1[l])
        h = xx @ W1[l].T + b1[l]
        h = 0.5 * h * (1 + erf(h / np.sqrt(2)))
        xx = ln(xx + (h @ W2[l].T + b2[l]), g2[l], be2[l])
    return xx[:, 0, :].reshape(B, T, D)
